# revision 1
# baseline (speedup 1.0000x reference)
"""Minibatch-discrimination kernel for Trainium2 (8 NeuronCores, SPMD), v2.

Math: M = einsum('nf,fbc->nbc', x, T); d[i,j,b] = sum_c |M[i,b,c]-M[j,b,c]|;
out[i,b] = sum_j exp(-d[i,j,b]) - 1; return concat([x, out], axis=1).

v2 exploits d(i,j)=d(j,i): core k only computes pairs against local j in
[0, 2560) (own block, gaps 1-3, gap 4) instead of [0, 4096):
  - j in [0, 512): diagonal block, upper triangle only (a step-mask matmul
    adds +25 to the psum for j<=i, killing exp); row sums cover j>i, column
    sums cover i<j. The self term is never computed, so no -1 at the end.
  - j in [512, 2048): gaps 1-3, full; row sums for own rows + column sums
    emitted as partial outputs for blocks k+1..k+3 (host adds them).
  - j in [2048, 2560): gap 4, row sums only (both endpoint cores compute
    their own rows against the partner block).
Host combines row parts + rotated column parts.

The pairwise pass is split across three engines (tunable per-window):
  - DVE tensor_scalar max (4x fp16): ad = max(mt_j, mt_i)
  - DVE scalar_tensor_tensor: acc = max(mt_j, mt_i) + ad_prev (merges two
    chunks into one matmul rhs, 2x mode)
  - ScalarE relu: R = relu(mt_j - mt_i); |a| = a + 2*relu(-a) flips that
    chunk's U_i sign in the exp bias: bias = U_maxch - U_reluch.
PE sums partition pairs via a 0/1 selector (K=128 -> M=64) into PSUM
([128=(i2,64b), jw] tiles), adds -V_j/2 via a K=64 matmul, ScalarE computes
exp(-2P + bias) with the j-sum via accum_out, and column sums accumulate
over all pr in persistent PSUM tiles via a ones-pair selector matmul over
the fp16 exp tile.
"""

import os
from contextlib import ExitStack

import numpy as np

N, F, B, C = 4096, 256, 64, 8
NCORES = 8
ROWS = N // NCORES          # 512 output rows per core
JDOM = ROWS * 5             # 2560: local j domain (diag + gaps 1-3 + gap 4)
COLW = ROWS * 4             # 2048: j range with column-sum partials
BC = B * C                  # 512
NCHUNK = BC // 128          # 4 partition-chunks of M.T
NPAIR = ROWS // 2           # 256 (two i's fill one 128-partition psum tile)
RELUCH = 3                  # chunk computed by ScalarE relu on window A1

_CACHE = {}


def _build_program():
    import concourse.bacc as bacc
    import concourse.tile as tile
    from concourse import mybir
    from concourse._compat import get_trn_type

    f32 = mybir.dt.float32
    f32r = mybir.dt.float32r
    fp16 = mybir.dt.float16
    Alu = mybir.AluOpType
    Act = mybir.ActivationFunctionType

    nc = bacc.Bacc(
        get_trn_type() or "TRN2",
        target_bir_lowering=False,
        debug=False,
        enable_asserts=True,
        num_devices=NCORES,
    )

    x_d = nc.dram_tensor("x_rot", [JDOM, F], f32, kind="ExternalInput").ap()
    t_d = nc.dram_tensor("t_mat", [F, BC], f32, kind="ExternalInput").ap()
    s_d = nc.dram_tensor("s_sel", [128, B], fp16, kind="ExternalInput").ap()
    sn_d = nc.dram_tensor("s_neg", [128, B], fp16, kind="ExternalInput").ap()
    sc_d = nc.dram_tensor("s_colw", [128, B], fp16, kind="ExternalInput").ap()
    id_d = nc.dram_tensor("ident", [128, 128], f32, kind="ExternalInput").ap()
    e2_d = nc.dram_tensor("e2_neg", [64, 128], f32, kind="ExternalInput").ap()
    bw_d = nc.dram_tensor("bw_mask", [32, 128], fp16, kind="ExternalInput").ap()
    st_d = nc.dram_tensor("stepb", [32, 512], fp16, kind="ExternalInput").ap()
    or_d = nc.dram_tensor("out_row", [ROWS, B], f32, kind="ExternalOutput").ap()
    oc_d = nc.dram_tensor("out_col", [B, COLW], f32, kind="ExternalOutput").ap()

    KCH = F // 128  # 2

    with tile.TileContext(nc) as tc, ExitStack() as ctx:
        singles = ctx.enter_context(tc.tile_pool(name="singles", bufs=1))
        xin = ctx.enter_context(tc.tile_pool(name="xin", bufs=2))
        psum = ctx.enter_context(tc.tile_pool(name="psum", bufs=3, space="PSUM"))
        colp = ctx.enter_context(tc.tile_pool(name="colp", bufs=1, space="PSUM"))
        adp_s = ctx.enter_context(tc.tile_pool(name="adp_s", bufs=2))
        adp = ctx.enter_context(tc.tile_pool(name="adp", bufs=3))
        escr_p = ctx.enter_context(tc.tile_pool(name="escr", bufs=3))

        # ---- constants -----------------------------------------------------
        s_sel = singles.tile([128, B], fp16)
        nc.sync.dma_start(out=s_sel, in_=s_d)
        s_neg = singles.tile([128, B], fp16)
        nc.sync.dma_start(out=s_neg, in_=sn_d)
        s_colw = singles.tile([128, B], fp16)
        nc.sync.dma_start(out=s_colw, in_=sc_d)
        ident = singles.tile([128, 128], f32)
        nc.sync.dma_start(out=ident, in_=id_d)
        e2_neg = singles.tile([64, 128], f32)
        nc.sync.dma_start(out=e2_neg, in_=e2_d)
        bw_sb = singles.tile([32, 128], fp16)
        nc.sync.dma_start(out=bw_sb, in_=bw_d)
        stepb = singles.tile([32, 512], fp16)
        nc.sync.dma_start(out=stepb, in_=st_d)

        # ---- T (already column-permuted on host) in sbuf: [k, bc] ----------
        t_sb = [singles.tile([128, BC], f32, tag=f"tsb{kc}", name=f"tsb{kc}")
                for kc in range(KCH)]
        t_v = t_d.rearrange("(kc p) q -> kc p q", p=128)
        for kc in range(KCH):
            nc.sync.dma_start(out=t_sb[kc], in_=t_v[kc])

        # ---- MT = (x @ T).T as 4 chunks [128, JDOM]; x transposed on the
        # fly per 512-j block through small rotating buffers
        mt_bf = [singles.tile([128, JDOM], fp16, tag=f"mtb{ch}", name=f"mtb{ch}")
                 for ch in range(NCHUNK)]
        x_v = x_d.rearrange("(t p) f -> t p f", p=128)  # 20 x [128, 256]
        for jt in range(JDOM // 512):
            xTj = [xin.tile([128, 512], f32, tag=f"xTj{kc}", name=f"xTj{kc}")
                   for kc in range(KCH)]
            for t in range(4):
                xt_in = xin.tile([128, F], f32, tag="xtile")
                nc.sync.dma_start(out=xt_in, in_=x_v[jt * 4 + t])
                for kc in range(KCH):
                    pt = psum.tile([128, 1024], f32, tag="ps")
                    nc.tensor.transpose(
                        pt[:, 0:128], xt_in[:, kc * 128:(kc + 1) * 128], ident
                    )
                    nc.scalar.copy(
                        out=xTj[kc][:, t * 128:(t + 1) * 128], in_=pt[:, 0:128]
                    )
            for ch in range(NCHUNK):
                pm = psum.tile([128, 1024], f32, tag="ps")
                for kc in range(KCH):
                    nc.tensor.matmul(
                        pm[:, 0:512],
                        t_sb[kc][:, ch * 128:(ch + 1) * 128],
                        xTj[kc],
                        start=(kc == 0),
                        stop=(kc == KCH - 1),
                    )
                # psum -> fp16 (this rounding defines the kernel's M)
                nc.vector.tensor_copy(
                    out=mt_bf[ch][:, jt * 512:(jt + 1) * 512], in_=pm[:, 0:512]
                )

        # ---- fp32 roundtrips of own-row M values (scalar operands must be
        # f32; equals the fp16 value exactly) + negated copy for relu bias
        mt_f32 = [singles.tile([128, ROWS], f32, tag=f"mtf{ch}", name=f"mtf{ch}")
                  for ch in range(NCHUNK)]
        for ch in range(NCHUNK):
            nc.scalar.copy(out=mt_f32[ch], in_=mt_bf[ch][:, 0:ROWS])
        mtn_f32 = singles.tile([128, ROWS], f32)
        nc.scalar.mul(out=mtn_f32, in_=mt_bf[RELUCH][:, 0:ROWS], mul=-1.0)

        # ---- VT[b, j] = sum_c M[j, b, c] for all local j; f32r for the
        # -V/2 correction matmuls
        vt32 = singles.tile([64, JDOM], f32r)
        e2_r = singles.tile([64, 128], f32r)
        nc.scalar.copy(out=e2_r, in_=e2_neg)
        for jt in range(JDOM // 512):
            pv = psum.tile([128, 1024], f32, tag="ps")
            for ch in range(NCHUNK):
                nc.tensor.matmul(
                    pv[0:64, 0:512],
                    s_sel,
                    mt_bf[ch][:, jt * 512:(jt + 1) * 512],
                    start=(ch == 0),
                    stop=(ch == NCHUNK - 1),
                )
            nc.scalar.copy(
                out=vt32[:, jt * 512:(jt + 1) * 512], in_=pv[0:64, 0:512]
            )

        # ---- u_all[p=(i2,b), pr] = U[2pr+i2, b] and u_mix (U_maxch -
        # U_reluch, the bias for the relu window), both built directly in the
        # bias layout via stride-2-column selector matmuls (a DRAM scatter
        # roundtrip here costs ~250us of full-pipeline stall)
        u_all = singles.tile([128, NPAIR], f32)
        u_mix = singles.tile([128, NPAIR], f32)
        for dst, negch in ((u_all, -1), (u_mix, RELUCH)):
            up = psum.tile([128, 1024], f32, tag="ps")
            for i2 in range(2):
                for ch in range(NCHUNK):
                    mv = mt_bf[ch][:, 0:ROWS].rearrange(
                        "p (pr two) -> p two pr", two=2)
                    nc.tensor.matmul(
                        up[i2 * 64:(i2 + 1) * 64, 0:NPAIR],
                        s_neg if ch == negch else s_sel,
                        mv[:, i2:i2 + 1, :],
                        start=(ch == 0),
                        stop=(ch == NCHUNK - 1),
                        skip_group_check=True,
                    )
            nc.scalar.copy(out=dst, in_=up[:, 0:NPAIR])

        # ---- per-(i,b) row partial sums: col = pr*3 + window ----------------
        psbuf = singles.tile([128, NPAIR * 3], f32)

        # ---- persistent column-sum accumulators (live across the pr loop);
        # both packed into one [128, 1024] psum tile: window A0 sums on
        # partitions 0-63, window A1 on 64-127 (matmul tile_position derives
        # from out.base_partition)
        colt = colp.tile([128, 1024], f32)
        colacc = [colt[0:64, :], colt[64:128, :]]

        # ---- main loop ------------------------------------------------------
        # windows: A0 = [0,1024) (diag+gap1a, step mask, col sums),
        #          A1 = [1024,2048) (gaps, col sums, relu chunk),
        #          B  = [2048,2560) (gap 4, row sums only)
        for pr in range(NPAIR):
            i0 = 2 * pr
            sk = min((i0 // 128) * 128, 384)
            # -- produce pairwise tiles for both rows of the pair ------------
            rhs = {}  # (i2, window) -> list of (tile, joff) rhs sources
            for i2 in range(2):
                i = i0 + i2
                # ch0/ch1: TS max over [sk, 2560), merged by a TT add (TT is
                # 2x fp16 mode; scalar_tensor_tensor measured 1x -> avoided)
                ad0 = adp_s.tile([128, JDOM], fp16, tag="ad0")
                nc.vector.tensor_scalar(
                    out=ad0[:, sk:JDOM],
                    in0=mt_bf[0][:, sk:JDOM],
                    scalar1=mt_f32[0][:, i:i + 1],
                    scalar2=None,
                    op0=Alu.max,
                )
                ad1 = adp_s.tile([128, JDOM], fp16, tag="ad1")
                nc.vector.tensor_scalar(
                    out=ad1[:, sk:JDOM],
                    in0=mt_bf[1][:, sk:JDOM],
                    scalar1=mt_f32[1][:, i:i + 1],
                    scalar2=None,
                    op0=Alu.max,
                )
                m01 = adp.tile([128, JDOM], fp16, tag="m01")
                nc.vector.tensor_tensor(
                    out=m01[:, sk:JDOM],
                    in0=ad0[:, sk:JDOM],
                    in1=ad1[:, sk:JDOM],
                    op=Alu.add,
                )
                # ch2: TS max over [sk, 2560)
                ad2 = adp.tile([128, JDOM], fp16, tag="ad2")
                nc.vector.tensor_scalar(
                    out=ad2[:, sk:JDOM],
                    in0=mt_bf[2][:, sk:JDOM],
                    scalar1=mt_f32[2][:, i:i + 1],
                    scalar2=None,
                    op0=Alu.max,
                )
                # ch3: TS max on A0 + B windows; ScalarE relu on A1
                ad3 = adp.tile([128, JDOM], fp16, tag="ad3")
                nc.vector.tensor_scalar(
                    out=ad3[:, sk:1024],
                    in0=mt_bf[3][:, sk:1024],
                    scalar1=mt_f32[3][:, i:i + 1],
                    scalar2=None,
                    op0=Alu.max,
                )
                nc.vector.tensor_scalar(
                    out=ad3[:, 2048:JDOM],
                    in0=mt_bf[3][:, 2048:JDOM],
                    scalar1=mt_f32[3][:, i:i + 1],
                    scalar2=None,
                    op0=Alu.max,
                )
                r3 = escr_p.tile([128, 1024], fp16, tag="r3")
                nc.scalar.activation(
                    out=r3,
                    in_=mt_bf[RELUCH][:, 1024:2048],
                    func=Act.Relu,
                    scale=1.0,
                    bias=mtn_f32[:, i:i + 1],
                )
                rhs[(i2, 0)] = [(m01, 0), (ad2, 0), (ad3, 0)]
                rhs[(i2, 1)] = [(m01, 0), (ad2, 0), (r3, 1024)]
                rhs[(i2, 2)] = [(m01, 0), (ad2, 0), (ad3, 0)]

            # -- windows -----------------------------------------------------
            for w, (j0, j1) in enumerate([(0, 1024), (1024, 2048), (2048, JDOM)]):
                jb0 = j0 + (sk if w == 0 else 0)
                dps = psum.tile([128, 1024], f32, tag="ps")
                # bank-aligned 512-col slices of [jb0, j1)
                js_chunks = []
                s = jb0
                while s < j1:
                    e = min((s // 512 + 1) * 512, j1)
                    js_chunks.append((s, e))
                    s = e
                # main selector matmuls
                for i2 in range(2):
                    for (js0, js1) in js_chunks:
                        for ri, (rt, roff) in enumerate(rhs[(i2, w)]):
                            nc.tensor.matmul(
                                dps[i2 * 64:(i2 + 1) * 64,
                                    js0 - j0:js1 - j0],
                                s_sel,
                                rt[:, js0 - roff:js1 - roff],
                                start=(ri == 0),
                                stop=False,
                                skip_group_check=True,
                            )
                # -V/2 correction
                for (js0, js1) in js_chunks:
                    nc.tensor.matmul(
                        dps[:, js0 - j0:js1 - j0],
                        e2_r,
                        vt32[:, js0:js1],
                        start=False,
                        stop=True,
                        skip_group_check=True,
                    )
                # step mask on A0: +25 for j <= i (covers the self term)
                if w == 0:
                    q = i0 - sk          # 0..126
                    nm = 128
                    off = 128 - q
                    nc.tensor.matmul(
                        dps[:, sk:sk + nm],
                        bw_sb,
                        stepb[:, off:off + nm],
                        start=False,
                        stop=True,
                        skip_group_check=True,
                    )
                # exp(-2P + bias), row sums via accum_out
                escr = escr_p.tile([128, 1024], fp16, tag="escr")
                nc.scalar.activation(
                    out=escr[:, jb0 - j0:j1 - j0],
                    in_=dps[:, jb0 - j0:j1 - j0],
                    func=Act.Exp,
                    scale=-2.0,
                    bias=(u_mix if w == 1 else u_all)[:, pr:pr + 1],
                    accum_out=psbuf[:, pr * 3 + w:pr * 3 + w + 1],
                )
                # column sums (A windows only): accumulate over all pr.
                # NOTE: start=True zeroes the whole 2KB psum bank, so each
                # bank of colacc must see exactly one start (its first MM).
                if w < 2:
                    if w == 0:
                        # bank 0: only [sk, 512) is valid escr this pr; cols
                        # below sk were completed by earlier prs
                        regions = [(sk, 512), (512, 1024)]
                    else:
                        regions = [(0, 512), (512, 1024)]
                    for (c0, c1) in regions:
                        nc.tensor.matmul(
                            colacc[w][:, c0:c1],
                            s_colw,
                            escr[:, c0:c1],
                            start=(pr == 0),
                            stop=(pr == NPAIR - 1),
                            skip_group_check=True,
                        )

        # ---- finish: row part ----------------------------------------------
        red = singles.tile([128, NPAIR], f32)
        tmp = singles.tile([128, NPAIR], f32)
        pv3 = psbuf.rearrange("p (c w) -> p c w", w=3)
        nc.vector.tensor_tensor(
            out=tmp, in0=pv3[:, :, 0], in1=pv3[:, :, 1], op=Alu.add
        )
        nc.vector.tensor_tensor(
            out=red, in0=tmp, in1=pv3[:, :, 2], op=Alu.add
        )
        # red[:, pr]: partition = i2*64 + b. Transpose 128-blocks so the DMA
        # descriptors are contiguous 256B runs.
        o_v = or_d.rearrange("(pr i2) b -> pr i2 b", i2=2)
        for blk in range(NPAIR // 128):
            pt = psum.tile([128, 1024], f32, tag="ps")
            nc.tensor.transpose(
                pt[:, 0:128], red[:, blk * 128:(blk + 1) * 128], ident
            )
            ot = xin.tile([128, 128], f32, tag="otile")
            nc.scalar.copy(out=ot, in_=pt[:, 0:128])
            ot_v = ot.rearrange("q (i2 b) -> q i2 b", i2=2)
            nc.sync.dma_start(out=o_v[blk * 128:(blk + 1) * 128], in_=ot_v)

        # ---- finish: column part (partition-aligned copy, remap in the DMA:
        # partitions (w b), free j -> out_col[b, w*1024 + j])
        col_sb = singles.tile([128, 1024], f32)
        nc.scalar.copy(out=col_sb, in_=colt)
        nc.sync.dma_start(out=oc_d[:, 0:1024], in_=col_sb[0:64, :])
        nc.sync.dma_start(out=oc_d[:, 1024:2048], in_=col_sb[64:128, :])

    nc.compile()
    return nc


def _get_program():
    if "nc" not in _CACHE:
        _CACHE["nc"] = _build_program()
    return _CACHE["nc"]


def _host_consts():
    s_sel = (np.arange(128)[:, None] // 2 == np.arange(B)[None, :]).astype(
        np.float16
    )
    s_neg = -s_sel
    s_colw = (np.arange(128)[:, None] % 64 == np.arange(B)[None, :]).astype(
        np.float16
    )
    ident = np.eye(128, dtype=np.float32)
    e2_neg = (-0.5 * (np.arange(64)[:, None] == (np.arange(128)[None, :] % 64))
              ).astype(np.float32)
    bw_mask = np.zeros((32, 128), dtype=np.float16)
    bw_mask[0, :64] = 25.0
    bw_mask[1, 64:] = 25.0
    stepb = np.zeros((32, 512), dtype=np.float16)
    stepb[0, : 128 + 1] = 1.0
    stepb[1, : 129 + 1] = 1.0
    return s_sel, s_neg, s_colw, ident, e2_neg, bw_mask, stepb


def _host_inputs(x, T):
    x = np.ascontiguousarray(x, dtype=np.float32)
    # permute T columns: q = ch*128 + b*2 + e  <->  (b, c=2ch+e)
    t_mat = np.ascontiguousarray(
        T.reshape(F, B, NCHUNK, 2).transpose(0, 2, 1, 3).reshape(F, BC),
        dtype=np.float32,
    )
    return x, t_mat


TRACE = bool(int(os.environ.get("KERNEL_TRACE", "0")))
LAST_RESULTS = None


def _make_ntff_hook():
    # the image's antenv lacks axon_hooks, but the injected libaxon_pjrt.so
    # carries the NTFF profile C ABI — drive it via ctypes directly
    import contextlib
    import ctypes

    so_path = "/opt/axon/libaxon_pjrt.so"
    if not os.path.exists(so_path):
        return None
    lib = ctypes.CDLL(so_path)
    if not hasattr(lib, "axon_start_nrt_profile"):
        return None
    lib.axon_start_nrt_profile.argtypes = [
        ctypes.POINTER(ctypes.c_int64),
        ctypes.c_size_t,
    ]
    lib.axon_start_nrt_profile.restype = ctypes.c_int64
    lib.axon_stop_nrt_profile.argtypes = [ctypes.c_char_p]
    lib.axon_stop_nrt_profile.restype = ctypes.c_int64

    @contextlib.contextmanager
    def _hook(output_dir, device_ids):
        import jax

        jax.devices()
        if device_ids:
            ids = (ctypes.c_int64 * len(device_ids))(*device_ids)
            rc = lib.axon_start_nrt_profile(ids, len(device_ids))
        else:
            rc = lib.axon_start_nrt_profile(None, 0)
        if rc != 0:
            raise RuntimeError(f"axon_start_nrt_profile rc={rc}")
        try:
            yield
        finally:
            n = lib.axon_stop_nrt_profile(str(output_dir).encode())
            print(f"profile: {n} file(s) written to {output_dir}")

    return _hook


def _ensure_axon_hook_stub():
    import importlib
    import sys
    import types

    try:
        importlib.import_module("antenv.axon_hooks")
    except ModuleNotFoundError:
        stub = types.ModuleType("antenv.axon_hooks")
        stub.get_axon_ntff_profile_hook = _make_ntff_hook
        sys.modules["antenv.axon_hooks"] = stub


def kernel(x: np.ndarray, T: np.ndarray) -> np.ndarray:
    global LAST_RESULTS
    _ensure_axon_hook_stub()
    from concourse.bass_utils import run_bass_kernel_spmd

    nc = _get_program()
    x, t_mat = _host_inputs(x, T)
    s_sel, s_neg, s_colw, ident, e2_neg, bw_mask, stepb = _host_consts()

    in_maps = []
    for k in range(NCORES):
        x_rot = np.roll(x, -ROWS * k, axis=0)[:JDOM] if k else x[:JDOM]
        in_maps.append(
            {"x_rot": np.ascontiguousarray(x_rot), "t_mat": t_mat,
             "s_sel": s_sel, "s_neg": s_neg, "s_colw": s_colw,
             "ident": ident, "e2_neg": e2_neg, "bw_mask": bw_mask,
             "stepb": stepb}
        )

    res = run_bass_kernel_spmd(
        nc, in_maps, core_ids=list(range(NCORES)), trace=TRACE
    )
    LAST_RESULTS = res
    out = np.zeros((N, B), dtype=np.float64)
    for k in range(NCORES):
        out[k * ROWS:(k + 1) * ROWS] += res.results[k]["out_row"].astype(
            np.float64
        )
        colp = res.results[k]["out_col"].astype(np.float64).T  # [COLW, B]
        gidx = (k * ROWS + np.arange(COLW)) % N
        np.add.at(out, gidx, colp)
    return np.concatenate([x, out.astype(np.float32)], axis=1)



# revision 2
# speedup vs baseline: 1.0218x; 1.0218x over previous
"""Minibatch-discrimination kernel for Trainium2 (8 NeuronCores, SPMD), v10.

Math: M = einsum('nf,fbc->nbc', x, T); d[i,j,b] = sum_c |M[i,b,c]-M[j,b,c]|;
out[i,b] = sum_j exp(-d[i,j,b]) - 1; return concat([x, out], axis=1).

Work split (exploits d(i,j)=d(j,i)): core k computes pairs against local j
in [0, 2560) of its rotated row domain instead of [0, 4096):
  - j in [0, 512): diagonal block, upper triangle only (a step-mask matmul
    adds +25 to the psum for j<=i, killing exp); row sums cover j>i, column
    sums cover i<j. The self term is never computed, so no -1 at the end.
  - j in [512, 2048): gaps 1-3, full; row sums for own rows + column sums
    emitted as partial outputs for blocks k+1..k+3 (host adds them).
  - j in [2048+256h, 2304+256h), h = (row >= 256): HALF of gap 4; the host
    swaps the gap-4 halves of x_rot on cores 4-7 so the two endpoint cores
    of each block pair cover complementary quadrants, and gap-4 emits
    column sums too (no duplicated work).
Host combines row parts + permuted column parts.

Engine split, per row pair (trace-balanced: PE 97%, DVE 89%, ScalarE 84%):
  - DVE: TS max at 4x fp16 for chunks 0-2 (full range) + ch3 (diag+gap4);
    no TT merges (they made DVE the wall at 96% busy).
  - ScalarE: relu for ch3 on window A1 (|a| = a + 2relu(-a), sign flip
    folded into the exp bias u_mix), plus the three window exps with row
    sums via accum_out.
  - PE: 4 selector rhs streams per row (0/1 selector, K=128 -> M=64),
    issued i2-interleaved so the two 64-col PE column-group chains run
    concurrently; -V_j/2 correction in fp16; column sums accumulate over
    all pr in persistent PSUM, emitted ONE PR LATE so the in-order PE
    queue never stalls on an exp.
"""

import os
from contextlib import ExitStack

import numpy as np

N, F, B, C = 4096, 256, 64, 8
NCORES = 8
ROWS = N // NCORES          # 512 output rows per core
JDOM = ROWS * 5             # 2560: local j domain (diag + gaps 1-3 + gap 4)
COLW = JDOM                 # 2560: j range with column-sum partials (v6:
                            # gap-4 emits col sums too)
BC = B * C                  # 512
NCHUNK = BC // 128          # 4 partition-chunks of M.T
NPAIR = ROWS // 2           # 256 (two i's fill one 128-partition psum tile)
RELUCH = 3                  # chunk computed by ScalarE relu on window A1
GH = 256                    # gap-4 half width: row half h pairs with
                            # partner cols [2048+GH*h, 2304+GH*h)

_CACHE = {}


def _build_program():
    import concourse.bacc as bacc
    import concourse.tile as tile
    from concourse import mybir
    from concourse._compat import get_trn_type

    f32 = mybir.dt.float32
    f32r = mybir.dt.float32r
    fp16 = mybir.dt.float16
    Alu = mybir.AluOpType
    Act = mybir.ActivationFunctionType

    nc = bacc.Bacc(
        get_trn_type() or "TRN2",
        target_bir_lowering=False,
        debug=False,
        enable_asserts=True,
        num_devices=NCORES,
    )

    x_d = nc.dram_tensor("x_rot", [JDOM, F], f32, kind="ExternalInput").ap()
    t_d = nc.dram_tensor("t_mat", [F, BC], f32, kind="ExternalInput").ap()
    s_d = nc.dram_tensor("s_sel", [128, B], fp16, kind="ExternalInput").ap()
    sn_d = nc.dram_tensor("s_neg", [128, B], fp16, kind="ExternalInput").ap()
    sc_d = nc.dram_tensor("s_colw", [128, B], fp16, kind="ExternalInput").ap()
    id_d = nc.dram_tensor("ident", [128, 128], f32, kind="ExternalInput").ap()
    e2_d = nc.dram_tensor("e2_neg", [64, 128], fp16, kind="ExternalInput").ap()
    bw_d = nc.dram_tensor("bw_mask", [32, 128], fp16, kind="ExternalInput").ap()
    st_d = nc.dram_tensor("stepb", [32, 512], fp16, kind="ExternalInput").ap()
    or_d = nc.dram_tensor("out_row", [ROWS, B], f32, kind="ExternalOutput").ap()
    oc_d = nc.dram_tensor("out_col", [B, COLW], f32, kind="ExternalOutput").ap()

    KCH = F // 128  # 2

    with tile.TileContext(nc) as tc, ExitStack() as ctx:
        singles = ctx.enter_context(tc.tile_pool(name="singles", bufs=1))
        xin = ctx.enter_context(tc.tile_pool(name="xin", bufs=2))
        psum = ctx.enter_context(tc.tile_pool(name="psum", bufs=2, space="PSUM"))
        psum_b = ctx.enter_context(tc.tile_pool(name="psum_b", bufs=1, space="PSUM"))
        colp = ctx.enter_context(tc.tile_pool(name="colp", bufs=1, space="PSUM"))
        adp = ctx.enter_context(tc.tile_pool(name="adp", bufs=3))
        escr_p = ctx.enter_context(tc.tile_pool(name="escr", bufs=4))

        # ---- constants -----------------------------------------------------
        s_sel = singles.tile([128, B], fp16)
        nc.sync.dma_start(out=s_sel, in_=s_d)
        s_neg = singles.tile([128, B], fp16)
        nc.sync.dma_start(out=s_neg, in_=sn_d)
        s_colw = singles.tile([128, B], fp16)
        nc.sync.dma_start(out=s_colw, in_=sc_d)
        ident = singles.tile([128, 128], f32)
        nc.sync.dma_start(out=ident, in_=id_d)
        e2_neg = singles.tile([64, 128], fp16)
        nc.sync.dma_start(out=e2_neg, in_=e2_d)
        bw_sb = singles.tile([32, 128], fp16)
        nc.sync.dma_start(out=bw_sb, in_=bw_d)
        stepb = singles.tile([32, 512], fp16)
        nc.sync.dma_start(out=stepb, in_=st_d)

        # ---- T (already column-permuted on host) in sbuf: [k, bc] ----------
        t_sb = [singles.tile([128, BC], f32, tag=f"tsb{kc}", name=f"tsb{kc}")
                for kc in range(KCH)]
        t_v = t_d.rearrange("(kc p) q -> kc p q", p=128)
        for kc in range(KCH):
            nc.sync.dma_start(out=t_sb[kc], in_=t_v[kc])

        # ---- MT = (x @ T).T as 4 chunks [128, JDOM]; x transposed on the
        # fly per 512-j block through small rotating buffers
        mt_bf = [singles.tile([128, JDOM], fp16, tag=f"mtb{ch}", name=f"mtb{ch}")
                 for ch in range(NCHUNK)]
        x_v = x_d.rearrange("(t p) f -> t p f", p=128)  # 20 x [128, 256]
        for jt in range(JDOM // 512):
            xTj = [xin.tile([128, 512], f32, tag=f"xTj{kc}", name=f"xTj{kc}")
                   for kc in range(KCH)]
            for t in range(4):
                xt_in = xin.tile([128, F], f32, tag="xtile")
                nc.sync.dma_start(out=xt_in, in_=x_v[jt * 4 + t])
                for kc in range(KCH):
                    pt = psum.tile([128, 1024], f32, tag="ps")
                    nc.tensor.transpose(
                        pt[:, 0:128], xt_in[:, kc * 128:(kc + 1) * 128], ident
                    )
                    nc.scalar.copy(
                        out=xTj[kc][:, t * 128:(t + 1) * 128], in_=pt[:, 0:128]
                    )
            for ch in range(NCHUNK):
                pm = psum.tile([128, 1024], f32, tag="ps")
                for kc in range(KCH):
                    nc.tensor.matmul(
                        pm[:, 0:512],
                        t_sb[kc][:, ch * 128:(ch + 1) * 128],
                        xTj[kc],
                        start=(kc == 0),
                        stop=(kc == KCH - 1),
                    )
                # psum -> fp16 (this rounding defines the kernel's M)
                nc.vector.tensor_copy(
                    out=mt_bf[ch][:, jt * 512:(jt + 1) * 512], in_=pm[:, 0:512]
                )

        # ---- fp32 roundtrips of own-row M values (scalar operands must be
        # f32; equals the fp16 value exactly) + negated copy for relu bias
        mt_f32 = [singles.tile([128, ROWS], f32, tag=f"mtf{ch}", name=f"mtf{ch}")
                  for ch in range(NCHUNK)]
        for ch in range(NCHUNK):
            nc.scalar.copy(out=mt_f32[ch], in_=mt_bf[ch][:, 0:ROWS])
        mtn_f32 = singles.tile([128, ROWS], f32)
        nc.scalar.mul(out=mtn_f32, in_=mt_bf[RELUCH][:, 0:ROWS], mul=-1.0)

        # ---- VT[b, j] = sum_c M[j, b, c] for all local j; fp16 so the
        # -V/2 correction matmul streams at 1 cycle/col
        vt16 = singles.tile([64, JDOM], fp16)
        for jt in range(JDOM // 512):
            pv = psum.tile([128, 1024], f32, tag="ps")
            for ch in range(NCHUNK):
                nc.tensor.matmul(
                    pv[0:64, 0:512],
                    s_sel,
                    mt_bf[ch][:, jt * 512:(jt + 1) * 512],
                    start=(ch == 0),
                    stop=(ch == NCHUNK - 1),
                )
            nc.scalar.copy(
                out=vt16[:, jt * 512:(jt + 1) * 512], in_=pv[0:64, 0:512]
            )

        # ---- u_all[p=(i2,b), pr] = U[2pr+i2, b] and u_mix (U_maxch -
        # U_reluch, the bias for the relu window), both built directly in the
        # bias layout via stride-2-column selector matmuls (a DRAM scatter
        # roundtrip here costs ~250us of full-pipeline stall)
        u_all = singles.tile([128, NPAIR], f32)
        u_mix = singles.tile([128, NPAIR], f32)
        for dst, negch in ((u_all, -1), (u_mix, RELUCH)):
            up = psum.tile([128, 1024], f32, tag="ps")
            for i2 in range(2):
                for ch in range(NCHUNK):
                    mv = mt_bf[ch][:, 0:ROWS].rearrange(
                        "p (pr two) -> p two pr", two=2)
                    nc.tensor.matmul(
                        up[i2 * 64:(i2 + 1) * 64, 0:NPAIR],
                        s_neg if ch == negch else s_sel,
                        mv[:, i2:i2 + 1, :],
                        start=(ch == 0),
                        stop=(ch == NCHUNK - 1),
                        skip_group_check=True,
                    )
            nc.scalar.copy(out=dst, in_=up[:, 0:NPAIR])

        # ---- per-(i,b) row partial sums: col = pr*3 + window ----------------
        psbuf = singles.tile([128, NPAIR * 3], f32)

        # ---- persistent column-sum accumulators (live across the pr loop);
        # both packed into one [128, 1024] psum tile: window A0 sums on
        # partitions 0-63, window A1 on 64-127 (matmul tile_position derives
        # from out.base_partition)
        colt = colp.tile([128, 1536], f32)
        colacc = [colt[0:64, 0:1024], colt[64:128, 0:1024]]
        colacc_b = colt[0:64, 1024:1536]

        pending_colsums = []
        # ---- main loop ------------------------------------------------------
        # windows: A0 = [sk,1024) (diag+gap1a, step mask, col sums),
        #          A1 = [1024,2048) (gaps, col sums, relu chunk),
        #          B  = [2048+GH*h, 2304+GH*h) (gap-4 half; row + col sums).
        # Row half h of each core pairs with partner-block half h; the host
        # swaps the gap-4 halves of x_rot on cores 4-7 so the two endpoint
        # cores of each block pair cover complementary quadrants.
        for pr in range(NPAIR):
            i0 = 2 * pr
            sk = min((i0 // 128) * 128, 384)
            h = 1 if pr >= NPAIR // 2 else 0
            gb0, gb1 = 2048 + GH * h, 2304 + GH * h
            ad_end = gb1
            # -- produce pairwise tiles for both rows of the pair ------------
            rhs = {}  # (i2, window) -> list of (tile, joff) rhs sources
            ads = {}
            for ch in range(3):
                for i2 in range(2):
                    i = i0 + i2
                    ad = adp.tile([128, JDOM], fp16, tag=f"ad{ch}_{i2}")
                    nc.vector.tensor_scalar(
                        out=ad[:, sk:ad_end],
                        in0=mt_bf[ch][:, sk:ad_end],
                        scalar1=mt_f32[ch][:, i:i + 1],
                        scalar2=None,
                        op0=Alu.max,
                    )
                    ads[(ch, i2)] = ad
            for i2 in range(2):
                i = i0 + i2
                # ch3: TS max on the A0 + B windows; ScalarE relu on A1
                ad3 = adp.tile([128, JDOM], fp16, tag=f"ad3_{i2}")
                nc.vector.tensor_scalar(
                    out=ad3[:, sk:1024],
                    in0=mt_bf[3][:, sk:1024],
                    scalar1=mt_f32[3][:, i:i + 1],
                    scalar2=None,
                    op0=Alu.max,
                )
                nc.vector.tensor_scalar(
                    out=ad3[:, gb0:gb1],
                    in0=mt_bf[3][:, gb0:gb1],
                    scalar1=mt_f32[3][:, i:i + 1],
                    scalar2=None,
                    op0=Alu.max,
                )
                r3 = escr_p.tile([128, 1024], fp16, tag="r3")
                nc.scalar.activation(
                    out=r3,
                    in_=mt_bf[RELUCH][:, 1024:2048],
                    func=Act.Relu,
                    scale=1.0,
                    bias=mtn_f32[:, i:i + 1],
                )
                a0, a1, a2 = ads[(0, i2)], ads[(1, i2)], ads[(2, i2)]
                rhs[(i2, 0)] = [(a0, 0), (a1, 0), (a2, 0), (ad3, 0)]
                rhs[(i2, 1)] = [(a0, 0), (a1, 0), (a2, 0), (r3, 1024)]
                rhs[(i2, 2)] = [(a0, 0), (a1, 0), (a2, 0), (ad3, 0)]

            # -- windows; colsum matmuls are pipelined one pr late so the
            # in-order PE queue never waits on an exp ------------------------
            colsum_args = []
            for w, (j0, j1) in enumerate([(0, 1024), (1024, 2048), (gb0, gb1)]):
                jb0 = j0 + (sk if w == 0 else 0)
                if w == 2:
                    dps = psum_b.tile([128, 512], f32, tag="psb")
                    jbase = 2048
                else:
                    dps = psum.tile([128, 1024], f32, tag="ps")
                    jbase = j0
                # bank-aligned 512-col slices of [jb0, j1)
                js_chunks = []
                s = jb0
                while s < j1:
                    e = min((s // 512 + 1) * 512, j1)
                    js_chunks.append((s, e))
                    s = e
                # main selector matmuls, i2-interleaved so the two PE
                # column-group chains (psum partitions 0-63 / 64-127) overlap
                for (js0, js1) in js_chunks:
                    for ri in range(4):
                        for i2 in range(2):
                            rt, roff = rhs[(i2, w)][ri]
                            nc.tensor.matmul(
                                dps[i2 * 64:(i2 + 1) * 64,
                                    js0 - jbase:js1 - jbase],
                                s_sel,
                                rt[:, js0 - roff:js1 - roff],
                                start=(ri == 0),
                                stop=False,
                                skip_group_check=True,
                            )
                # -V/2 correction
                for (js0, js1) in js_chunks:
                    nc.tensor.matmul(
                        dps[:, js0 - jbase:js1 - jbase],
                        e2_neg,
                        vt16[:, js0:js1],
                        start=False,
                        stop=True,
                        skip_group_check=True,
                    )
                # step mask on A0: +25 for j <= i (covers the self term)
                if w == 0:
                    q = i0 - sk          # 0..126
                    nm = 128
                    off = 128 - q
                    nc.tensor.matmul(
                        dps[:, sk:sk + nm],
                        bw_sb,
                        stepb[:, off:off + nm],
                        start=False,
                        stop=True,
                        skip_group_check=True,
                    )
                # exp(-2P + bias), row sums via accum_out
                if w == 2:
                    escr = escr_p.tile([128, 512], fp16, tag="escrb")
                else:
                    escr = escr_p.tile([128, 1024], fp16, tag="escr")
                nc.scalar.activation(
                    out=escr[:, jb0 - jbase:j1 - jbase],
                    in_=dps[:, jb0 - jbase:j1 - jbase],
                    func=Act.Exp,
                    scale=-2.0,
                    bias=(u_mix if w == 1 else u_all)[:, pr:pr + 1],
                    accum_out=psbuf[:, pr * 3 + w:pr * 3 + w + 1],
                )
                # column sums: accumulate over all pr into persistent psum.
                # NOTE: start=True zeroes the written partitions' whole psum
                # bank, so each bank of colacc must see exactly one start.
                if w < 2:
                    regions = [(sk, 512), (512, 1024)] if w == 0 else \
                              [(0, 512), (512, 1024)]
                    for ridx, (c0, c1) in enumerate(regions):
                        colsum_args.append(
                            (ridx, w, colacc[w][:, c0:c1], escr[:, c0:c1])
                        )
                else:
                    colsum_args.append(
                        (2, 2, colacc_b[:, GH * h:GH * h + 256],
                         escr[:, GH * h:GH * h + 256])
                    )
            # order: (A0 r0 | A1 r0) pair, (A0 r1 | A1 r1) pair, then B --
            # adjacent MMs hit different PE column groups and overlap
            colsum_args.sort(key=lambda a: (a[0], a[1]))
            for _, _, dst, src in pending_colsums:
                nc.tensor.matmul(
                    dst,
                    s_colw,
                    src,
                    start=(pr == 1),
                    stop=False,
                    skip_group_check=True,
                )
            pending_colsums = colsum_args

        for _, _, dst, src in pending_colsums:
            nc.tensor.matmul(
                dst,
                s_colw,
                src,
                start=False,
                stop=True,
                skip_group_check=True,
            )

        # ---- finish: row part ----------------------------------------------
        red = singles.tile([128, NPAIR], f32)
        tmp = singles.tile([128, NPAIR], f32)
        pv3 = psbuf.rearrange("p (c w) -> p c w", w=3)
        nc.vector.tensor_tensor(
            out=tmp, in0=pv3[:, :, 0], in1=pv3[:, :, 1], op=Alu.add
        )
        nc.vector.tensor_tensor(
            out=red, in0=tmp, in1=pv3[:, :, 2], op=Alu.add
        )
        # red[:, pr]: partition = i2*64 + b. Transpose 128-blocks so the DMA
        # descriptors are contiguous 256B runs.
        o_v = or_d.rearrange("(pr i2) b -> pr i2 b", i2=2)
        for blk in range(NPAIR // 128):
            pt = psum.tile([128, 1024], f32, tag="ps")
            nc.tensor.transpose(
                pt[:, 0:128], red[:, blk * 128:(blk + 1) * 128], ident
            )
            ot = xin.tile([128, 128], f32, tag="otile")
            nc.scalar.copy(out=ot, in_=pt[:, 0:128])
            ot_v = ot.rearrange("q (i2 b) -> q i2 b", i2=2)
            nc.sync.dma_start(out=o_v[blk * 128:(blk + 1) * 128], in_=ot_v)

        # ---- finish: column part (partition-aligned copy, remap in the DMA:
        # partitions (w b), free j -> out_col[b, w*1024 + j]; gap-4 block
        # rides cols [1024, 1536) of partitions 0-63)
        col_sb = singles.tile([128, 1536], f32)
        nc.scalar.copy(out=col_sb, in_=colt)
        nc.sync.dma_start(out=oc_d[:, 0:1024], in_=col_sb[0:64, 0:1024])
        nc.sync.dma_start(out=oc_d[:, 1024:2048], in_=col_sb[64:128, 0:1024])
        nc.sync.dma_start(out=oc_d[:, 2048:2560], in_=col_sb[0:64, 1024:1536])

    nc.compile()
    return nc


def _get_program():
    if "nc" not in _CACHE:
        _CACHE["nc"] = _build_program()
    return _CACHE["nc"]


def _host_consts():
    s_sel = (np.arange(128)[:, None] // 2 == np.arange(B)[None, :]).astype(
        np.float16
    )
    s_neg = -s_sel
    s_colw = (np.arange(128)[:, None] % 64 == np.arange(B)[None, :]).astype(
        np.float16
    )
    ident = np.eye(128, dtype=np.float32)
    e2_neg = (-0.5 * (np.arange(64)[:, None] == (np.arange(128)[None, :] % 64))
              ).astype(np.float16)
    bw_mask = np.zeros((32, 128), dtype=np.float16)
    bw_mask[0, :64] = 25.0
    bw_mask[1, 64:] = 25.0
    stepb = np.zeros((32, 512), dtype=np.float16)
    stepb[0, : 128 + 1] = 1.0
    stepb[1, : 129 + 1] = 1.0
    return s_sel, s_neg, s_colw, ident, e2_neg, bw_mask, stepb


def _host_inputs(x, T):
    x = np.ascontiguousarray(x, dtype=np.float32)
    # permute T columns: q = ch*128 + b*2 + e  <->  (b, c=2ch+e)
    t_mat = np.ascontiguousarray(
        T.reshape(F, B, NCHUNK, 2).transpose(0, 2, 1, 3).reshape(F, BC),
        dtype=np.float32,
    )
    return x, t_mat


TRACE = bool(int(os.environ.get("KERNEL_TRACE", "0")))
LAST_RESULTS = None


def _make_ntff_hook():
    # the image's antenv lacks axon_hooks, but the injected libaxon_pjrt.so
    # carries the NTFF profile C ABI — drive it via ctypes directly
    import contextlib
    import ctypes

    so_path = "/opt/axon/libaxon_pjrt.so"
    if not os.path.exists(so_path):
        return None
    lib = ctypes.CDLL(so_path)
    if not hasattr(lib, "axon_start_nrt_profile"):
        return None
    lib.axon_start_nrt_profile.argtypes = [
        ctypes.POINTER(ctypes.c_int64),
        ctypes.c_size_t,
    ]
    lib.axon_start_nrt_profile.restype = ctypes.c_int64
    lib.axon_stop_nrt_profile.argtypes = [ctypes.c_char_p]
    lib.axon_stop_nrt_profile.restype = ctypes.c_int64

    @contextlib.contextmanager
    def _hook(output_dir, device_ids):
        import jax

        jax.devices()
        if device_ids:
            ids = (ctypes.c_int64 * len(device_ids))(*device_ids)
            rc = lib.axon_start_nrt_profile(ids, len(device_ids))
        else:
            rc = lib.axon_start_nrt_profile(None, 0)
        if rc != 0:
            raise RuntimeError(f"axon_start_nrt_profile rc={rc}")
        try:
            yield
        finally:
            n = lib.axon_stop_nrt_profile(str(output_dir).encode())
            print(f"profile: {n} file(s) written to {output_dir}")

    return _hook


def _ensure_axon_hook_stub():
    import importlib
    import sys
    import types

    try:
        importlib.import_module("antenv.axon_hooks")
    except ModuleNotFoundError:
        stub = types.ModuleType("antenv.axon_hooks")
        stub.get_axon_ntff_profile_hook = _make_ntff_hook
        sys.modules["antenv.axon_hooks"] = stub


def kernel(x: np.ndarray, T: np.ndarray) -> np.ndarray:
    global LAST_RESULTS
    _ensure_axon_hook_stub()
    from concourse.bass_utils import run_bass_kernel_spmd

    nc = _get_program()
    x, t_mat = _host_inputs(x, T)
    s_sel, s_neg, s_colw, ident, e2_neg, bw_mask, stepb = _host_consts()

    in_maps = []
    for k in range(NCORES):
        x_rot = np.array(np.roll(x, -ROWS * k, axis=0)[:JDOM] if k else x[:JDOM])
        if k >= NCORES // 2:
            # swap the gap-4 halves so the two endpoint cores of each block
            # pair cover complementary row/col quadrants
            g = x_rot[2048:2560].copy()
            x_rot[2048:2304] = g[256:512]
            x_rot[2304:2560] = g[0:256]
        in_maps.append(
            {"x_rot": np.ascontiguousarray(x_rot), "t_mat": t_mat,
             "s_sel": s_sel, "s_neg": s_neg, "s_colw": s_colw,
             "ident": ident, "e2_neg": e2_neg, "bw_mask": bw_mask,
             "stepb": stepb}
        )

    res = run_bass_kernel_spmd(
        nc, in_maps, core_ids=list(range(NCORES)), trace=TRACE
    )
    LAST_RESULTS = res
    out = np.zeros((N, B), dtype=np.float64)
    for k in range(NCORES):
        out[k * ROWS:(k + 1) * ROWS] += res.results[k]["out_row"].astype(
            np.float64
        )
        colp = res.results[k]["out_col"].astype(np.float64).T  # [COLW, B]
        gidx = (k * ROWS + np.arange(COLW)) % N
        if k >= NCORES // 2:
            gidx = gidx.copy()
            gidx[2048:2304] = (k * ROWS + np.arange(2304, 2560)) % N
            gidx[2304:2560] = (k * ROWS + np.arange(2048, 2304)) % N
        np.add.at(out, gidx, colp)
    return np.concatenate([x, out.astype(np.float32)], axis=1)



# revision 3
# speedup vs baseline: 1.0364x; 1.0143x over previous
"""Minibatch-discrimination kernel for Trainium2 (8 NeuronCores, SPMD), v11.

Math: M = einsum('nf,fbc->nbc', x, T); d[i,j,b] = sum_c |M[i,b,c]-M[j,b,c]|;
out[i,b] = sum_j exp(-d[i,j,b]) - 1; return concat([x, out], axis=1).

Work split (exploits d(i,j)=d(j,i)): core k computes pairs against local j
in [0, 2560) of its rotated row domain instead of [0, 4096):
  - j in [0, 512): diagonal block, upper triangle only; a DVE memset
    plants +30 in ad0 for j<=i so exp dies there; row sums cover j>i,
    column sums cover i<j. The self term is never computed, so no -1.
  - j in [512, 2048): gaps 1-3, full; row sums for own rows + column sums
    emitted as partial outputs for blocks k+1..k+3 (host adds them).
  - j in [2048+256h, 2304+256h), h = (row >= 256): HALF of gap 4; the host
    swaps the gap-4 halves of x_rot on cores 4-7 so the two endpoint cores
    of each block pair cover complementary quadrants, and gap-4 emits
    column sums too (no duplicated work).
Host combines row parts + permuted column parts.

Engine split, per row pair (trace-balanced: PE ~95%, DVE ~90%, ScalarE 84%):
  - DVE: TS max at 4x fp16 for chunks 0-2 (full range) + ch3 (diag+gap4),
    plus the diagonal-kill memsets; no TT merges (they made DVE the wall).
  - ScalarE: relu for ch3 on window A1 (|a| = a + 2relu(-a), sign flip
    folded into the exp bias u_mix), plus the three window exps with row
    sums via accum_out.
  - PE: 4 selector rhs streams per row (0/1 selector, K=128 -> M=64),
    issued i2-interleaved so the two 64-col PE column-group chains run
    concurrently; -V_j/2 correction in fp16; column sums accumulate over
    all pr in persistent PSUM, emitted ONE PR LATE so the in-order PE
    queue never stalls on an exp.
"""

import os
from contextlib import ExitStack

import numpy as np

N, F, B, C = 4096, 256, 64, 8
NCORES = 8
ROWS = N // NCORES          # 512 output rows per core
JDOM = ROWS * 5             # 2560: local j domain (diag + gaps 1-3 + gap 4)
COLW = JDOM                 # 2560: j range with column-sum partials (v6:
                            # gap-4 emits col sums too)
BC = B * C                  # 512
NCHUNK = BC // 128          # 4 partition-chunks of M.T
NPAIR = ROWS // 2           # 256 (two i's fill one 128-partition psum tile)
RELUCH = 3                  # chunk computed by ScalarE relu on window A1
GH = 256                    # gap-4 half width: row half h pairs with
                            # partner cols [2048+GH*h, 2304+GH*h)

_CACHE = {}


def _build_program():
    import concourse.bacc as bacc
    import concourse.tile as tile
    from concourse import mybir
    from concourse._compat import get_trn_type

    f32 = mybir.dt.float32
    f32r = mybir.dt.float32r
    fp16 = mybir.dt.float16
    Alu = mybir.AluOpType
    Act = mybir.ActivationFunctionType

    nc = bacc.Bacc(
        get_trn_type() or "TRN2",
        target_bir_lowering=False,
        debug=False,
        enable_asserts=True,
        num_devices=NCORES,
    )

    x_d = nc.dram_tensor("x_rot", [JDOM, F], f32, kind="ExternalInput").ap()
    t_d = nc.dram_tensor("t_mat", [F, BC], f32, kind="ExternalInput").ap()
    s_d = nc.dram_tensor("s_sel", [128, B], fp16, kind="ExternalInput").ap()
    sn_d = nc.dram_tensor("s_neg", [128, B], fp16, kind="ExternalInput").ap()
    sc_d = nc.dram_tensor("s_colw", [128, B], fp16, kind="ExternalInput").ap()
    id_d = nc.dram_tensor("ident", [128, 128], f32, kind="ExternalInput").ap()
    e2_d = nc.dram_tensor("e2_neg", [64, 128], fp16, kind="ExternalInput").ap()
    or_d = nc.dram_tensor("out_row", [ROWS, B], f32, kind="ExternalOutput").ap()
    oc_d = nc.dram_tensor("out_col", [B, COLW], f32, kind="ExternalOutput").ap()

    KCH = F // 128  # 2

    with tile.TileContext(nc) as tc, ExitStack() as ctx:
        singles = ctx.enter_context(tc.tile_pool(name="singles", bufs=1))
        xin = ctx.enter_context(tc.tile_pool(name="xin", bufs=2))
        psum = ctx.enter_context(tc.tile_pool(name="psum", bufs=2, space="PSUM"))
        psum_b = ctx.enter_context(tc.tile_pool(name="psum_b", bufs=1, space="PSUM"))
        colp = ctx.enter_context(tc.tile_pool(name="colp", bufs=1, space="PSUM"))
        adp = ctx.enter_context(tc.tile_pool(name="adp", bufs=3))
        escr_p = ctx.enter_context(tc.tile_pool(name="escr", bufs=4))

        # ---- constants -----------------------------------------------------
        s_sel = singles.tile([128, B], fp16)
        nc.sync.dma_start(out=s_sel, in_=s_d)
        s_neg = singles.tile([128, B], fp16)
        nc.sync.dma_start(out=s_neg, in_=sn_d)
        s_colw = singles.tile([128, B], fp16)
        nc.sync.dma_start(out=s_colw, in_=sc_d)
        ident = singles.tile([128, 128], f32)
        nc.sync.dma_start(out=ident, in_=id_d)
        e2_neg = singles.tile([64, 128], fp16)
        nc.sync.dma_start(out=e2_neg, in_=e2_d)

        # ---- T (already column-permuted on host) in sbuf: [k, bc] ----------
        t_sb = [singles.tile([128, BC], f32, tag=f"tsb{kc}", name=f"tsb{kc}")
                for kc in range(KCH)]
        t_v = t_d.rearrange("(kc p) q -> kc p q", p=128)
        for kc in range(KCH):
            nc.sync.dma_start(out=t_sb[kc], in_=t_v[kc])

        # ---- MT = (x @ T).T as 4 chunks [128, JDOM]; x transposed on the
        # fly per 512-j block through small rotating buffers
        mt_bf = [singles.tile([128, JDOM], fp16, tag=f"mtb{ch}", name=f"mtb{ch}")
                 for ch in range(NCHUNK)]
        x_v = x_d.rearrange("(t p) f -> t p f", p=128)  # 20 x [128, 256]
        for jt in range(JDOM // 512):
            xTj = [xin.tile([128, 512], f32, tag=f"xTj{kc}", name=f"xTj{kc}")
                   for kc in range(KCH)]
            for t in range(4):
                xt_in = xin.tile([128, F], f32, tag="xtile")
                nc.sync.dma_start(out=xt_in, in_=x_v[jt * 4 + t])
                for kc in range(KCH):
                    pt = psum.tile([128, 1024], f32, tag="ps")
                    nc.tensor.transpose(
                        pt[:, 0:128], xt_in[:, kc * 128:(kc + 1) * 128], ident
                    )
                    nc.scalar.copy(
                        out=xTj[kc][:, t * 128:(t + 1) * 128], in_=pt[:, 0:128]
                    )
            for ch in range(NCHUNK):
                pm = psum.tile([128, 1024], f32, tag="ps")
                for kc in range(KCH):
                    nc.tensor.matmul(
                        pm[:, 0:512],
                        t_sb[kc][:, ch * 128:(ch + 1) * 128],
                        xTj[kc],
                        start=(kc == 0),
                        stop=(kc == KCH - 1),
                    )
                # psum -> fp16 (this rounding defines the kernel's M)
                nc.vector.tensor_copy(
                    out=mt_bf[ch][:, jt * 512:(jt + 1) * 512], in_=pm[:, 0:512]
                )

        # ---- fp32 roundtrips of own-row M values (scalar operands must be
        # f32; equals the fp16 value exactly) + negated copy for relu bias
        mt_f32 = [singles.tile([128, ROWS], f32, tag=f"mtf{ch}", name=f"mtf{ch}")
                  for ch in range(NCHUNK)]
        for ch in range(NCHUNK):
            nc.scalar.copy(out=mt_f32[ch], in_=mt_bf[ch][:, 0:ROWS])
        mtn_f32 = singles.tile([128, ROWS], f32)
        nc.scalar.mul(out=mtn_f32, in_=mt_bf[RELUCH][:, 0:ROWS], mul=-1.0)

        # ---- VT[b, j] = sum_c M[j, b, c] for all local j; fp16 so the
        # -V/2 correction matmul streams at 1 cycle/col
        vt16 = singles.tile([64, JDOM], fp16)
        for jt in range(JDOM // 512):
            pv = psum.tile([128, 1024], f32, tag="ps")
            for ch in range(NCHUNK):
                nc.tensor.matmul(
                    pv[0:64, 0:512],
                    s_sel,
                    mt_bf[ch][:, jt * 512:(jt + 1) * 512],
                    start=(ch == 0),
                    stop=(ch == NCHUNK - 1),
                )
            nc.scalar.copy(
                out=vt16[:, jt * 512:(jt + 1) * 512], in_=pv[0:64, 0:512]
            )

        # ---- u_all[p=(i2,b), pr] = U[2pr+i2, b] and u_mix (U_maxch -
        # U_reluch, the bias for the relu window), both built directly in the
        # bias layout via stride-2-column selector matmuls (a DRAM scatter
        # roundtrip here costs ~250us of full-pipeline stall)
        u_all = singles.tile([128, NPAIR], f32)
        u_mix = singles.tile([128, NPAIR], f32)
        for dst, negch in ((u_all, -1), (u_mix, RELUCH)):
            up = psum.tile([128, 1024], f32, tag="ps")
            for i2 in range(2):
                for ch in range(NCHUNK):
                    mv = mt_bf[ch][:, 0:ROWS].rearrange(
                        "p (pr two) -> p two pr", two=2)
                    nc.tensor.matmul(
                        up[i2 * 64:(i2 + 1) * 64, 0:NPAIR],
                        s_neg if ch == negch else s_sel,
                        mv[:, i2:i2 + 1, :],
                        start=(ch == 0),
                        stop=(ch == NCHUNK - 1),
                        skip_group_check=True,
                    )
            nc.scalar.copy(out=dst, in_=up[:, 0:NPAIR])

        # ---- per-(i,b) row partial sums: col = pr*3 + window ----------------
        psbuf = singles.tile([128, NPAIR * 3], f32)

        # ---- persistent column-sum accumulators (live across the pr loop);
        # both packed into one [128, 1024] psum tile: window A0 sums on
        # partitions 0-63, window A1 on 64-127 (matmul tile_position derives
        # from out.base_partition)
        colt = colp.tile([128, 1536], f32)
        colacc = [colt[0:64, 0:1024], colt[64:128, 0:1024]]
        colacc_b = colt[0:64, 1024:1536]

        pending_colsums = []
        # ---- main loop ------------------------------------------------------
        # windows: A0 = [sk,1024) (diag+gap1a, step mask, col sums),
        #          A1 = [1024,2048) (gaps, col sums, relu chunk),
        #          B  = [2048+GH*h, 2304+GH*h) (gap-4 half; row + col sums).
        # Row half h of each core pairs with partner-block half h; the host
        # swaps the gap-4 halves of x_rot on cores 4-7 so the two endpoint
        # cores of each block pair cover complementary quadrants.
        for pr in range(NPAIR):
            i0 = 2 * pr
            sk = min((i0 // 128) * 128, 384)
            h = 1 if pr >= NPAIR // 2 else 0
            gb0, gb1 = 2048 + GH * h, 2304 + GH * h
            ad_end = gb1
            # -- produce pairwise tiles for both rows of the pair ------------
            rhs = {}  # (i2, window) -> list of (tile, joff) rhs sources
            ads = {}
            for ch in range(3):
                for i2 in range(2):
                    i = i0 + i2
                    ad = adp.tile([128, JDOM], fp16, tag=f"ad{ch}_{i2}")
                    nc.vector.tensor_scalar(
                        out=ad[:, sk:ad_end],
                        in0=mt_bf[ch][:, sk:ad_end],
                        scalar1=mt_f32[ch][:, i:i + 1],
                        scalar2=None,
                        op0=Alu.max,
                    )
                    if ch == 0:
                        # diagonal kill: j <= i gets ad0 = 30 so the pair
                        # sum contributes +60 to P and exp(-2P+bias) == 0
                        # (replaces the v2 step-mask matmul; worst case
                        # rest >= -27, bias <= +36 -> arg <= -30)
                        nc.vector.memset(ad[:, sk:i + 1], 30.0)
                    ads[(ch, i2)] = ad
            for i2 in range(2):
                i = i0 + i2
                # ch3: TS max on the A0 + B windows; ScalarE relu on A1
                ad3 = adp.tile([128, JDOM], fp16, tag=f"ad3_{i2}")
                nc.vector.tensor_scalar(
                    out=ad3[:, sk:1024],
                    in0=mt_bf[3][:, sk:1024],
                    scalar1=mt_f32[3][:, i:i + 1],
                    scalar2=None,
                    op0=Alu.max,
                )
                nc.vector.tensor_scalar(
                    out=ad3[:, gb0:gb1],
                    in0=mt_bf[3][:, gb0:gb1],
                    scalar1=mt_f32[3][:, i:i + 1],
                    scalar2=None,
                    op0=Alu.max,
                )
                r3 = escr_p.tile([128, 1024], fp16, tag="r3")
                nc.scalar.activation(
                    out=r3,
                    in_=mt_bf[RELUCH][:, 1024:2048],
                    func=Act.Relu,
                    scale=1.0,
                    bias=mtn_f32[:, i:i + 1],
                )
                a0, a1, a2 = ads[(0, i2)], ads[(1, i2)], ads[(2, i2)]
                rhs[(i2, 0)] = [(a0, 0), (a1, 0), (a2, 0), (ad3, 0)]
                rhs[(i2, 1)] = [(a0, 0), (a1, 0), (a2, 0), (r3, 1024)]
                rhs[(i2, 2)] = [(a0, 0), (a1, 0), (a2, 0), (ad3, 0)]

            # -- windows; colsum matmuls are pipelined one pr late so the
            # in-order PE queue never waits on an exp ------------------------
            colsum_args = []
            for w, (j0, j1) in enumerate([(0, 1024), (1024, 2048), (gb0, gb1)]):
                jb0 = j0 + (sk if w == 0 else 0)
                if w == 2:
                    dps = psum_b.tile([128, 512], f32, tag="psb")
                    jbase = 2048
                else:
                    dps = psum.tile([128, 1024], f32, tag="ps")
                    jbase = j0
                # bank-aligned 512-col slices of [jb0, j1)
                js_chunks = []
                s = jb0
                while s < j1:
                    e = min((s // 512 + 1) * 512, j1)
                    js_chunks.append((s, e))
                    s = e
                # main selector matmuls, i2-interleaved so the two PE
                # column-group chains (psum partitions 0-63 / 64-127) overlap
                for (js0, js1) in js_chunks:
                    for ri in range(4):
                        for i2 in range(2):
                            rt, roff = rhs[(i2, w)][ri]
                            nc.tensor.matmul(
                                dps[i2 * 64:(i2 + 1) * 64,
                                    js0 - jbase:js1 - jbase],
                                s_sel,
                                rt[:, js0 - roff:js1 - roff],
                                start=(ri == 0),
                                stop=False,
                                skip_group_check=True,
                            )
                # -V/2 correction
                for (js0, js1) in js_chunks:
                    nc.tensor.matmul(
                        dps[:, js0 - jbase:js1 - jbase],
                        e2_neg,
                        vt16[:, js0:js1],
                        start=False,
                        stop=True,
                        skip_group_check=True,
                    )
                # exp(-2P + bias), row sums via accum_out
                if w == 2:
                    escr = escr_p.tile([128, 512], fp16, tag="escrb")
                else:
                    escr = escr_p.tile([128, 1024], fp16, tag="escr")
                nc.scalar.activation(
                    out=escr[:, jb0 - jbase:j1 - jbase],
                    in_=dps[:, jb0 - jbase:j1 - jbase],
                    func=Act.Exp,
                    scale=-2.0,
                    bias=(u_mix if w == 1 else u_all)[:, pr:pr + 1],
                    accum_out=psbuf[:, pr * 3 + w:pr * 3 + w + 1],
                )
                # column sums: accumulate over all pr into persistent psum.
                # NOTE: start=True zeroes the written partitions' whole psum
                # bank, so each bank of colacc must see exactly one start.
                if w < 2:
                    regions = [(sk, 512), (512, 1024)] if w == 0 else \
                              [(0, 512), (512, 1024)]
                    for ridx, (c0, c1) in enumerate(regions):
                        colsum_args.append(
                            (ridx, w, colacc[w][:, c0:c1], escr[:, c0:c1])
                        )
                else:
                    colsum_args.append(
                        (2, 2, colacc_b[:, GH * h:GH * h + 256],
                         escr[:, GH * h:GH * h + 256])
                    )
            # order: (A0 r0 | A1 r0) pair, (A0 r1 | A1 r1) pair, then B --
            # adjacent MMs hit different PE column groups and overlap
            colsum_args.sort(key=lambda a: (a[0], a[1]))
            for _, _, dst, src in pending_colsums:
                nc.tensor.matmul(
                    dst,
                    s_colw,
                    src,
                    start=(pr == 1),
                    stop=False,
                    skip_group_check=True,
                )
            pending_colsums = colsum_args

        for _, _, dst, src in pending_colsums:
            nc.tensor.matmul(
                dst,
                s_colw,
                src,
                start=False,
                stop=True,
                skip_group_check=True,
            )

        # ---- finish: row part ----------------------------------------------
        red = singles.tile([128, NPAIR], f32)
        tmp = singles.tile([128, NPAIR], f32)
        pv3 = psbuf.rearrange("p (c w) -> p c w", w=3)
        nc.vector.tensor_tensor(
            out=tmp, in0=pv3[:, :, 0], in1=pv3[:, :, 1], op=Alu.add
        )
        nc.vector.tensor_tensor(
            out=red, in0=tmp, in1=pv3[:, :, 2], op=Alu.add
        )
        # red[:, pr]: partition = i2*64 + b. Transpose 128-blocks so the DMA
        # descriptors are contiguous 256B runs.
        o_v = or_d.rearrange("(pr i2) b -> pr i2 b", i2=2)
        for blk in range(NPAIR // 128):
            pt = psum.tile([128, 1024], f32, tag="ps")
            nc.tensor.transpose(
                pt[:, 0:128], red[:, blk * 128:(blk + 1) * 128], ident
            )
            ot = xin.tile([128, 128], f32, tag="otile")
            nc.scalar.copy(out=ot, in_=pt[:, 0:128])
            ot_v = ot.rearrange("q (i2 b) -> q i2 b", i2=2)
            nc.sync.dma_start(out=o_v[blk * 128:(blk + 1) * 128], in_=ot_v)

        # ---- finish: column part (partition-aligned copy, remap in the DMA:
        # partitions (w b), free j -> out_col[b, w*1024 + j]; gap-4 block
        # rides cols [1024, 1536) of partitions 0-63)
        col_sb = singles.tile([128, 1536], f32)
        nc.scalar.copy(out=col_sb, in_=colt)
        nc.sync.dma_start(out=oc_d[:, 0:1024], in_=col_sb[0:64, 0:1024])
        nc.sync.dma_start(out=oc_d[:, 1024:2048], in_=col_sb[64:128, 0:1024])
        nc.sync.dma_start(out=oc_d[:, 2048:2560], in_=col_sb[0:64, 1024:1536])

    nc.compile()
    return nc


def _get_program():
    if "nc" not in _CACHE:
        _CACHE["nc"] = _build_program()
    return _CACHE["nc"]


def _host_consts():
    s_sel = (np.arange(128)[:, None] // 2 == np.arange(B)[None, :]).astype(
        np.float16
    )
    s_neg = -s_sel
    s_colw = (np.arange(128)[:, None] % 64 == np.arange(B)[None, :]).astype(
        np.float16
    )
    ident = np.eye(128, dtype=np.float32)
    e2_neg = (-0.5 * (np.arange(64)[:, None] == (np.arange(128)[None, :] % 64))
              ).astype(np.float16)
    return s_sel, s_neg, s_colw, ident, e2_neg


def _host_inputs(x, T):
    x = np.ascontiguousarray(x, dtype=np.float32)
    # permute T columns: q = ch*128 + b*2 + e  <->  (b, c=2ch+e)
    t_mat = np.ascontiguousarray(
        T.reshape(F, B, NCHUNK, 2).transpose(0, 2, 1, 3).reshape(F, BC),
        dtype=np.float32,
    )
    return x, t_mat


TRACE = bool(int(os.environ.get("KERNEL_TRACE", "0")))
LAST_RESULTS = None


def _make_ntff_hook():
    # the image's antenv lacks axon_hooks, but the injected libaxon_pjrt.so
    # carries the NTFF profile C ABI — drive it via ctypes directly
    import contextlib
    import ctypes

    so_path = "/opt/axon/libaxon_pjrt.so"
    if not os.path.exists(so_path):
        return None
    lib = ctypes.CDLL(so_path)
    if not hasattr(lib, "axon_start_nrt_profile"):
        return None
    lib.axon_start_nrt_profile.argtypes = [
        ctypes.POINTER(ctypes.c_int64),
        ctypes.c_size_t,
    ]
    lib.axon_start_nrt_profile.restype = ctypes.c_int64
    lib.axon_stop_nrt_profile.argtypes = [ctypes.c_char_p]
    lib.axon_stop_nrt_profile.restype = ctypes.c_int64

    @contextlib.contextmanager
    def _hook(output_dir, device_ids):
        import jax

        jax.devices()
        if device_ids:
            ids = (ctypes.c_int64 * len(device_ids))(*device_ids)
            rc = lib.axon_start_nrt_profile(ids, len(device_ids))
        else:
            rc = lib.axon_start_nrt_profile(None, 0)
        if rc != 0:
            raise RuntimeError(f"axon_start_nrt_profile rc={rc}")
        try:
            yield
        finally:
            n = lib.axon_stop_nrt_profile(str(output_dir).encode())
            print(f"profile: {n} file(s) written to {output_dir}")

    return _hook


def _ensure_axon_hook_stub():
    import importlib
    import sys
    import types

    try:
        importlib.import_module("antenv.axon_hooks")
    except ModuleNotFoundError:
        stub = types.ModuleType("antenv.axon_hooks")
        stub.get_axon_ntff_profile_hook = _make_ntff_hook
        sys.modules["antenv.axon_hooks"] = stub


def kernel(x: np.ndarray, T: np.ndarray) -> np.ndarray:
    global LAST_RESULTS
    _ensure_axon_hook_stub()
    from concourse.bass_utils import run_bass_kernel_spmd

    nc = _get_program()
    x, t_mat = _host_inputs(x, T)
    s_sel, s_neg, s_colw, ident, e2_neg = _host_consts()

    in_maps = []
    for k in range(NCORES):
        x_rot = np.array(np.roll(x, -ROWS * k, axis=0)[:JDOM] if k else x[:JDOM])
        if k >= NCORES // 2:
            # swap the gap-4 halves so the two endpoint cores of each block
            # pair cover complementary row/col quadrants
            g = x_rot[2048:2560].copy()
            x_rot[2048:2304] = g[256:512]
            x_rot[2304:2560] = g[0:256]
        in_maps.append(
            {"x_rot": np.ascontiguousarray(x_rot), "t_mat": t_mat,
             "s_sel": s_sel, "s_neg": s_neg, "s_colw": s_colw,
             "ident": ident, "e2_neg": e2_neg}
        )

    res = run_bass_kernel_spmd(
        nc, in_maps, core_ids=list(range(NCORES)), trace=TRACE
    )
    LAST_RESULTS = res
    out = np.zeros((N, B), dtype=np.float64)
    for k in range(NCORES):
        out[k * ROWS:(k + 1) * ROWS] += res.results[k]["out_row"].astype(
            np.float64
        )
        colp = res.results[k]["out_col"].astype(np.float64).T  # [COLW, B]
        gidx = (k * ROWS + np.arange(COLW)) % N
        if k >= NCORES // 2:
            gidx = gidx.copy()
            gidx[2048:2304] = (k * ROWS + np.arange(2304, 2560)) % N
            gidx[2304:2560] = (k * ROWS + np.arange(2048, 2304)) % N
        np.add.at(out, gidx, colp)
    return np.concatenate([x, out.astype(np.float32)], axis=1)



# revision 4
# speedup vs baseline: 1.0464x; 1.0097x over previous
"""Minibatch-discrimination kernel for Trainium2 (8 NeuronCores, SPMD), v12.

Math: M = einsum('nf,fbc->nbc', x, T); d[i,j,b] = sum_c |M[i,b,c]-M[j,b,c]|;
out[i,b] = sum_j exp(-d[i,j,b]) - 1; return concat([x, out], axis=1).

Work split (exploits d(i,j)=d(j,i)): core k computes pairs against local j
in [0, 2560) of its rotated row domain instead of [0, 4096):
  - j in [0, 512): diagonal block, upper triangle only at 64-row
    granularity; a DVE memset plants +30 in ad0 for j<=i so exp dies
    there; row sums cover j>i, column sums cover i<j. The self term is
    never computed, so no -1 at the end.
  - j in [512, 2048): gaps 1-3, full; row sums for own rows + column sums
    emitted as partial outputs for blocks k+1..k+3 (host adds them).
  - j in [2048+256h, 2304+256h), h = (row >= 256): HALF of gap 4; the host
    swaps the gap-4 halves of x_rot on cores 4-7 so the two endpoint cores
    of each block pair cover complementary quadrants, and gap-4 emits
    column sums too (no duplicated work).
Host combines row parts + permuted column parts.

Engine split, per row pair (trace-balanced: PE ~97%, DVE ~92%, ScalarE 86%):
  - DVE: TS max at 4x fp16 for chunks 0-2 (full range) + ch3 (diag+gap4),
    plus the diagonal-kill memsets; no TT merges (they made DVE the wall).
  - ScalarE: relu for ch3 on window A1 (|a| = a + 2relu(-a), sign flip
    folded into the exp bias u_mix), plus the three window exps with row
    sums via accum_out.
  - PE: 4 selector rhs streams per row (0/1 selector, K=128 -> M=64),
    issued i2-interleaved so the two 64-col PE column-group chains run
    concurrently; -V_j/2 correction in fp16; column sums accumulate over
    all pr in persistent PSUM, emitted ONE PR LATE so the in-order PE
    queue never stalls on an exp.
"""

import os
from contextlib import ExitStack

import numpy as np

N, F, B, C = 4096, 256, 64, 8
NCORES = 8
ROWS = N // NCORES          # 512 output rows per core
JDOM = ROWS * 5             # 2560: local j domain (diag + gaps 1-3 + gap 4)
COLW = JDOM                 # 2560: j range with column-sum partials (v6:
                            # gap-4 emits col sums too)
BC = B * C                  # 512
NCHUNK = BC // 128          # 4 partition-chunks of M.T
NPAIR = ROWS // 2           # 256 (two i's fill one 128-partition psum tile)
RELUCH = 3                  # chunk computed by ScalarE relu on window A1
GH = 256                    # gap-4 half width: row half h pairs with
                            # partner cols [2048+GH*h, 2304+GH*h)

_CACHE = {}


def _build_program():
    import concourse.bacc as bacc
    import concourse.tile as tile
    from concourse import mybir
    from concourse._compat import get_trn_type

    f32 = mybir.dt.float32
    f32r = mybir.dt.float32r
    fp16 = mybir.dt.float16
    Alu = mybir.AluOpType
    Act = mybir.ActivationFunctionType

    nc = bacc.Bacc(
        get_trn_type() or "TRN2",
        target_bir_lowering=False,
        debug=False,
        enable_asserts=True,
        num_devices=NCORES,
    )

    x_d = nc.dram_tensor("x_rot", [JDOM, F], f32, kind="ExternalInput").ap()
    t_d = nc.dram_tensor("t_mat", [F, BC], f32, kind="ExternalInput").ap()
    s_d = nc.dram_tensor("s_sel", [128, B], fp16, kind="ExternalInput").ap()
    sn_d = nc.dram_tensor("s_neg", [128, B], fp16, kind="ExternalInput").ap()
    sc_d = nc.dram_tensor("s_colw", [128, B], fp16, kind="ExternalInput").ap()
    id_d = nc.dram_tensor("ident", [128, 128], f32, kind="ExternalInput").ap()
    e2_d = nc.dram_tensor("e2_neg", [64, 128], fp16, kind="ExternalInput").ap()
    or_d = nc.dram_tensor("out_row", [ROWS, B], f32, kind="ExternalOutput").ap()
    oc_d = nc.dram_tensor("out_col", [B, COLW], f32, kind="ExternalOutput").ap()

    KCH = F // 128  # 2

    with tile.TileContext(nc) as tc, ExitStack() as ctx:
        singles = ctx.enter_context(tc.tile_pool(name="singles", bufs=1))
        xin = ctx.enter_context(tc.tile_pool(name="xin", bufs=2))
        psum = ctx.enter_context(tc.tile_pool(name="psum", bufs=2, space="PSUM"))
        psum_b = ctx.enter_context(tc.tile_pool(name="psum_b", bufs=1, space="PSUM"))
        colp = ctx.enter_context(tc.tile_pool(name="colp", bufs=1, space="PSUM"))
        adp = ctx.enter_context(tc.tile_pool(name="adp", bufs=3))
        escr_p = ctx.enter_context(tc.tile_pool(name="escr", bufs=4))

        # ---- constants -----------------------------------------------------
        s_sel = singles.tile([128, B], fp16)
        nc.sync.dma_start(out=s_sel, in_=s_d)
        s_neg = singles.tile([128, B], fp16)
        nc.sync.dma_start(out=s_neg, in_=sn_d)
        s_colw = singles.tile([128, B], fp16)
        nc.sync.dma_start(out=s_colw, in_=sc_d)
        ident = singles.tile([128, 128], f32)
        nc.sync.dma_start(out=ident, in_=id_d)
        e2_neg = singles.tile([64, 128], fp16)
        nc.sync.dma_start(out=e2_neg, in_=e2_d)

        # ---- T (already column-permuted on host) in sbuf: [k, bc] ----------
        t_sb = [singles.tile([128, BC], f32, tag=f"tsb{kc}", name=f"tsb{kc}")
                for kc in range(KCH)]
        t_v = t_d.rearrange("(kc p) q -> kc p q", p=128)
        for kc in range(KCH):
            nc.sync.dma_start(out=t_sb[kc], in_=t_v[kc])

        # ---- MT = (x @ T).T as 4 chunks [128, JDOM]; x transposed on the
        # fly per 512-j block through small rotating buffers
        mt_bf = [singles.tile([128, JDOM], fp16, tag=f"mtb{ch}", name=f"mtb{ch}")
                 for ch in range(NCHUNK)]
        x_v = x_d.rearrange("(t p) f -> t p f", p=128)  # 20 x [128, 256]
        for jt in range(JDOM // 512):
            xTj = [xin.tile([128, 512], f32, tag=f"xTj{kc}", name=f"xTj{kc}")
                   for kc in range(KCH)]
            for t in range(4):
                xt_in = xin.tile([128, F], f32, tag="xtile")
                nc.sync.dma_start(out=xt_in, in_=x_v[jt * 4 + t])
                for kc in range(KCH):
                    pt = psum.tile([128, 1024], f32, tag="ps")
                    nc.tensor.transpose(
                        pt[:, 0:128], xt_in[:, kc * 128:(kc + 1) * 128], ident
                    )
                    nc.scalar.copy(
                        out=xTj[kc][:, t * 128:(t + 1) * 128], in_=pt[:, 0:128]
                    )
            for ch in range(NCHUNK):
                pm = psum.tile([128, 1024], f32, tag="ps")
                for kc in range(KCH):
                    nc.tensor.matmul(
                        pm[:, 0:512],
                        t_sb[kc][:, ch * 128:(ch + 1) * 128],
                        xTj[kc],
                        start=(kc == 0),
                        stop=(kc == KCH - 1),
                    )
                # psum -> fp16 (this rounding defines the kernel's M)
                nc.vector.tensor_copy(
                    out=mt_bf[ch][:, jt * 512:(jt + 1) * 512], in_=pm[:, 0:512]
                )

        # ---- fp32 roundtrips of own-row M values (scalar operands must be
        # f32; equals the fp16 value exactly) + negated copy for relu bias
        mt_f32 = [singles.tile([128, ROWS], f32, tag=f"mtf{ch}", name=f"mtf{ch}")
                  for ch in range(NCHUNK)]
        for ch in range(NCHUNK):
            nc.scalar.copy(out=mt_f32[ch], in_=mt_bf[ch][:, 0:ROWS])
        mtn_f32 = singles.tile([128, ROWS], f32)
        nc.scalar.mul(out=mtn_f32, in_=mt_bf[RELUCH][:, 0:ROWS], mul=-1.0)

        # ---- VT[b, j] = sum_c M[j, b, c] for all local j; fp16 so the
        # -V/2 correction matmul streams at 1 cycle/col
        vt16 = singles.tile([64, JDOM], fp16)
        for jt in range(JDOM // 512):
            pv = psum.tile([128, 1024], f32, tag="ps")
            for ch in range(NCHUNK):
                nc.tensor.matmul(
                    pv[0:64, 0:512],
                    s_sel,
                    mt_bf[ch][:, jt * 512:(jt + 1) * 512],
                    start=(ch == 0),
                    stop=(ch == NCHUNK - 1),
                )
            nc.scalar.copy(
                out=vt16[:, jt * 512:(jt + 1) * 512], in_=pv[0:64, 0:512]
            )

        # ---- u_all[p=(i2,b), pr] = U[2pr+i2, b] and u_mix (U_maxch -
        # U_reluch, the bias for the relu window), both built directly in the
        # bias layout via stride-2-column selector matmuls (a DRAM scatter
        # roundtrip here costs ~250us of full-pipeline stall)
        u_all = singles.tile([128, NPAIR], f32)
        u_mix = singles.tile([128, NPAIR], f32)
        for dst, negch in ((u_all, -1), (u_mix, RELUCH)):
            up = psum.tile([128, 1024], f32, tag="ps")
            for i2 in range(2):
                for ch in range(NCHUNK):
                    mv = mt_bf[ch][:, 0:ROWS].rearrange(
                        "p (pr two) -> p two pr", two=2)
                    nc.tensor.matmul(
                        up[i2 * 64:(i2 + 1) * 64, 0:NPAIR],
                        s_neg if ch == negch else s_sel,
                        mv[:, i2:i2 + 1, :],
                        start=(ch == 0),
                        stop=(ch == NCHUNK - 1),
                        skip_group_check=True,
                    )
            nc.scalar.copy(out=dst, in_=up[:, 0:NPAIR])

        # ---- per-(i,b) row partial sums: col = pr*3 + window ----------------
        psbuf = singles.tile([128, NPAIR * 3], f32)

        # ---- persistent column-sum accumulators (live across the pr loop);
        # both packed into one [128, 1024] psum tile: window A0 sums on
        # partitions 0-63, window A1 on 64-127 (matmul tile_position derives
        # from out.base_partition)
        colt = colp.tile([128, 1536], f32)
        colacc = [colt[0:64, 0:1024], colt[64:128, 0:1024]]
        colacc_b = colt[0:64, 1024:1536]

        pending_colsums = []
        # ---- main loop ------------------------------------------------------
        # windows: A0 = [sk,1024) (diag+gap1a, step mask, col sums),
        #          A1 = [1024,2048) (gaps, col sums, relu chunk),
        #          B  = [2048+GH*h, 2304+GH*h) (gap-4 half; row + col sums).
        # Row half h of each core pairs with partner-block half h; the host
        # swaps the gap-4 halves of x_rot on cores 4-7 so the two endpoint
        # cores of each block pair cover complementary quadrants.
        for pr in range(NPAIR):
            i0 = 2 * pr
            sk = min((i0 // 64) * 64, 448)
            h = 1 if pr >= NPAIR // 2 else 0
            gb0, gb1 = 2048 + GH * h, 2304 + GH * h
            ad_end = gb1
            # -- produce pairwise tiles for both rows of the pair ------------
            rhs = {}  # (i2, window) -> list of (tile, joff) rhs sources
            ads = {}
            for ch in range(3):
                for i2 in range(2):
                    i = i0 + i2
                    ad = adp.tile([128, JDOM], fp16, tag=f"ad{ch}_{i2}")
                    nc.vector.tensor_scalar(
                        out=ad[:, sk:ad_end],
                        in0=mt_bf[ch][:, sk:ad_end],
                        scalar1=mt_f32[ch][:, i:i + 1],
                        scalar2=None,
                        op0=Alu.max,
                    )
                    if ch == 0:
                        # diagonal kill: j <= i gets ad0 = 30 so the pair
                        # sum contributes +60 to P and exp(-2P+bias) == 0
                        # (replaces the v2 step-mask matmul; worst case
                        # rest >= -27, bias <= +36 -> arg <= -30)
                        nc.vector.memset(ad[:, sk:i + 1], 30.0)
                    ads[(ch, i2)] = ad
            for i2 in range(2):
                i = i0 + i2
                # ch3: TS max on the A0 + B windows; ScalarE relu on A1
                ad3 = adp.tile([128, JDOM], fp16, tag=f"ad3_{i2}")
                nc.vector.tensor_scalar(
                    out=ad3[:, sk:1024],
                    in0=mt_bf[3][:, sk:1024],
                    scalar1=mt_f32[3][:, i:i + 1],
                    scalar2=None,
                    op0=Alu.max,
                )
                nc.vector.tensor_scalar(
                    out=ad3[:, gb0:gb1],
                    in0=mt_bf[3][:, gb0:gb1],
                    scalar1=mt_f32[3][:, i:i + 1],
                    scalar2=None,
                    op0=Alu.max,
                )
                r3 = escr_p.tile([128, 1024], fp16, tag="r3")
                nc.scalar.activation(
                    out=r3,
                    in_=mt_bf[RELUCH][:, 1024:2048],
                    func=Act.Relu,
                    scale=1.0,
                    bias=mtn_f32[:, i:i + 1],
                )
                a0, a1, a2 = ads[(0, i2)], ads[(1, i2)], ads[(2, i2)]
                rhs[(i2, 0)] = [(a0, 0), (a1, 0), (a2, 0), (ad3, 0)]
                rhs[(i2, 1)] = [(a0, 0), (a1, 0), (a2, 0), (r3, 1024)]
                rhs[(i2, 2)] = [(a0, 0), (a1, 0), (a2, 0), (ad3, 0)]

            # -- windows; colsum matmuls are pipelined one pr late so the
            # in-order PE queue never waits on an exp ------------------------
            colsum_args = []
            for w, (j0, j1) in enumerate([(0, 1024), (1024, 2048), (gb0, gb1)]):
                jb0 = j0 + (sk if w == 0 else 0)
                if w == 2:
                    dps = psum_b.tile([128, 512], f32, tag="psb")
                    jbase = 2048
                else:
                    dps = psum.tile([128, 1024], f32, tag="ps")
                    jbase = j0
                # bank-aligned 512-col slices of [jb0, j1)
                js_chunks = []
                s = jb0
                while s < j1:
                    e = min((s // 512 + 1) * 512, j1)
                    js_chunks.append((s, e))
                    s = e
                # main selector matmuls, i2-interleaved so the two PE
                # column-group chains (psum partitions 0-63 / 64-127) overlap
                for (js0, js1) in js_chunks:
                    for ri in range(4):
                        for i2 in range(2):
                            rt, roff = rhs[(i2, w)][ri]
                            nc.tensor.matmul(
                                dps[i2 * 64:(i2 + 1) * 64,
                                    js0 - jbase:js1 - jbase],
                                s_sel,
                                rt[:, js0 - roff:js1 - roff],
                                start=(ri == 0),
                                stop=False,
                                skip_group_check=True,
                            )
                # -V/2 correction
                for (js0, js1) in js_chunks:
                    nc.tensor.matmul(
                        dps[:, js0 - jbase:js1 - jbase],
                        e2_neg,
                        vt16[:, js0:js1],
                        start=False,
                        stop=True,
                        skip_group_check=True,
                    )
                # exp(-2P + bias), row sums via accum_out
                if w == 2:
                    escr = escr_p.tile([128, 512], fp16, tag="escrb")
                else:
                    escr = escr_p.tile([128, 1024], fp16, tag="escr")
                nc.scalar.activation(
                    out=escr[:, jb0 - jbase:j1 - jbase],
                    in_=dps[:, jb0 - jbase:j1 - jbase],
                    func=Act.Exp,
                    scale=-2.0,
                    bias=(u_mix if w == 1 else u_all)[:, pr:pr + 1],
                    accum_out=psbuf[:, pr * 3 + w:pr * 3 + w + 1],
                )
                # column sums: accumulate over all pr into persistent psum.
                # NOTE: start=True zeroes the written partitions' whole psum
                # bank, so each bank of colacc must see exactly one start.
                if w < 2:
                    regions = [(sk, 512), (512, 1024)] if w == 0 else \
                              [(0, 512), (512, 1024)]
                    for ridx, (c0, c1) in enumerate(regions):
                        colsum_args.append(
                            (ridx, w, colacc[w][:, c0:c1], escr[:, c0:c1])
                        )
                else:
                    colsum_args.append(
                        (2, 2, colacc_b[:, GH * h:GH * h + 256],
                         escr[:, GH * h:GH * h + 256])
                    )
            # order: (A0 r0 | A1 r0) pair, (A0 r1 | A1 r1) pair, then B --
            # adjacent MMs hit different PE column groups and overlap
            colsum_args.sort(key=lambda a: (a[0], a[1]))
            for _, _, dst, src in pending_colsums:
                nc.tensor.matmul(
                    dst,
                    s_colw,
                    src,
                    start=(pr == 1),
                    stop=False,
                    skip_group_check=True,
                )
            pending_colsums = colsum_args

        for _, _, dst, src in pending_colsums:
            nc.tensor.matmul(
                dst,
                s_colw,
                src,
                start=False,
                stop=True,
                skip_group_check=True,
            )

        # ---- finish: row part ----------------------------------------------
        red = singles.tile([128, NPAIR], f32)
        tmp = singles.tile([128, NPAIR], f32)
        pv3 = psbuf.rearrange("p (c w) -> p c w", w=3)
        nc.vector.tensor_tensor(
            out=tmp, in0=pv3[:, :, 0], in1=pv3[:, :, 1], op=Alu.add
        )
        nc.vector.tensor_tensor(
            out=red, in0=tmp, in1=pv3[:, :, 2], op=Alu.add
        )
        # red[:, pr]: partition = i2*64 + b. Transpose 128-blocks so the DMA
        # descriptors are contiguous 256B runs.
        o_v = or_d.rearrange("(pr i2) b -> pr i2 b", i2=2)
        for blk in range(NPAIR // 128):
            pt = psum.tile([128, 1024], f32, tag="ps")
            nc.tensor.transpose(
                pt[:, 0:128], red[:, blk * 128:(blk + 1) * 128], ident
            )
            ot = xin.tile([128, 128], f32, tag="otile")
            nc.scalar.copy(out=ot, in_=pt[:, 0:128])
            ot_v = ot.rearrange("q (i2 b) -> q i2 b", i2=2)
            nc.sync.dma_start(out=o_v[blk * 128:(blk + 1) * 128], in_=ot_v)

        # ---- finish: column part (partition-aligned copy, remap in the DMA:
        # partitions (w b), free j -> out_col[b, w*1024 + j]; gap-4 block
        # rides cols [1024, 1536) of partitions 0-63)
        col_sb = singles.tile([128, 1536], f32)
        nc.scalar.copy(out=col_sb, in_=colt)
        nc.sync.dma_start(out=oc_d[:, 0:1024], in_=col_sb[0:64, 0:1024])
        nc.sync.dma_start(out=oc_d[:, 1024:2048], in_=col_sb[64:128, 0:1024])
        nc.sync.dma_start(out=oc_d[:, 2048:2560], in_=col_sb[0:64, 1024:1536])

    nc.compile()
    return nc


def _get_program():
    if "nc" not in _CACHE:
        _CACHE["nc"] = _build_program()
    return _CACHE["nc"]


def _host_consts():
    s_sel = (np.arange(128)[:, None] // 2 == np.arange(B)[None, :]).astype(
        np.float16
    )
    s_neg = -s_sel
    s_colw = (np.arange(128)[:, None] % 64 == np.arange(B)[None, :]).astype(
        np.float16
    )
    ident = np.eye(128, dtype=np.float32)
    e2_neg = (-0.5 * (np.arange(64)[:, None] == (np.arange(128)[None, :] % 64))
              ).astype(np.float16)
    return s_sel, s_neg, s_colw, ident, e2_neg


def _host_inputs(x, T):
    x = np.ascontiguousarray(x, dtype=np.float32)
    # permute T columns: q = ch*128 + b*2 + e  <->  (b, c=2ch+e)
    t_mat = np.ascontiguousarray(
        T.reshape(F, B, NCHUNK, 2).transpose(0, 2, 1, 3).reshape(F, BC),
        dtype=np.float32,
    )
    return x, t_mat


TRACE = bool(int(os.environ.get("KERNEL_TRACE", "0")))
LAST_RESULTS = None


def _make_ntff_hook():
    # the image's antenv lacks axon_hooks, but the injected libaxon_pjrt.so
    # carries the NTFF profile C ABI — drive it via ctypes directly
    import contextlib
    import ctypes

    so_path = "/opt/axon/libaxon_pjrt.so"
    if not os.path.exists(so_path):
        return None
    lib = ctypes.CDLL(so_path)
    if not hasattr(lib, "axon_start_nrt_profile"):
        return None
    lib.axon_start_nrt_profile.argtypes = [
        ctypes.POINTER(ctypes.c_int64),
        ctypes.c_size_t,
    ]
    lib.axon_start_nrt_profile.restype = ctypes.c_int64
    lib.axon_stop_nrt_profile.argtypes = [ctypes.c_char_p]
    lib.axon_stop_nrt_profile.restype = ctypes.c_int64

    @contextlib.contextmanager
    def _hook(output_dir, device_ids):
        import jax

        jax.devices()
        if device_ids:
            ids = (ctypes.c_int64 * len(device_ids))(*device_ids)
            rc = lib.axon_start_nrt_profile(ids, len(device_ids))
        else:
            rc = lib.axon_start_nrt_profile(None, 0)
        if rc != 0:
            raise RuntimeError(f"axon_start_nrt_profile rc={rc}")
        try:
            yield
        finally:
            n = lib.axon_stop_nrt_profile(str(output_dir).encode())
            print(f"profile: {n} file(s) written to {output_dir}")

    return _hook


def _ensure_axon_hook_stub():
    import importlib
    import sys
    import types

    try:
        importlib.import_module("antenv.axon_hooks")
    except ModuleNotFoundError:
        stub = types.ModuleType("antenv.axon_hooks")
        stub.get_axon_ntff_profile_hook = _make_ntff_hook
        sys.modules["antenv.axon_hooks"] = stub


def kernel(x: np.ndarray, T: np.ndarray) -> np.ndarray:
    global LAST_RESULTS
    _ensure_axon_hook_stub()
    from concourse.bass_utils import run_bass_kernel_spmd

    nc = _get_program()
    x, t_mat = _host_inputs(x, T)
    s_sel, s_neg, s_colw, ident, e2_neg = _host_consts()

    in_maps = []
    for k in range(NCORES):
        x_rot = np.array(np.roll(x, -ROWS * k, axis=0)[:JDOM] if k else x[:JDOM])
        if k >= NCORES // 2:
            # swap the gap-4 halves so the two endpoint cores of each block
            # pair cover complementary row/col quadrants
            g = x_rot[2048:2560].copy()
            x_rot[2048:2304] = g[256:512]
            x_rot[2304:2560] = g[0:256]
        in_maps.append(
            {"x_rot": np.ascontiguousarray(x_rot), "t_mat": t_mat,
             "s_sel": s_sel, "s_neg": s_neg, "s_colw": s_colw,
             "ident": ident, "e2_neg": e2_neg}
        )

    res = run_bass_kernel_spmd(
        nc, in_maps, core_ids=list(range(NCORES)), trace=TRACE
    )
    LAST_RESULTS = res
    out = np.zeros((N, B), dtype=np.float64)
    for k in range(NCORES):
        out[k * ROWS:(k + 1) * ROWS] += res.results[k]["out_row"].astype(
            np.float64
        )
        colp = res.results[k]["out_col"].astype(np.float64).T  # [COLW, B]
        gidx = (k * ROWS + np.arange(COLW)) % N
        if k >= NCORES // 2:
            gidx = gidx.copy()
            gidx[2048:2304] = (k * ROWS + np.arange(2304, 2560)) % N
            gidx[2304:2560] = (k * ROWS + np.arange(2048, 2304)) % N
        np.add.at(out, gidx, colp)
    return np.concatenate([x, out.astype(np.float32)], axis=1)

